# revision 58
# baseline (speedup 1.0000x reference)
"""Trainium2 Bass kernel for an encoder block (B=8, S=1024, D=768, H=12, F=3072).

Sharding: data-parallel over batch — 8 batch elements onto 8 NeuronCores, no
collectives. Each core runs the full encoder block on its [S, D] slice.

Key optimizations vs a straightforward fp32r implementation:
- Mask compaction: the attention mask zeroes ~half the keys; the host gathers
  the unmasked key positions (padded to NK=640), so scores/exp/ctx run on 640
  instead of 1024 keys.
- fp8 DoubleRow matmuls (0.5 cycles/row, 2 K-tiles per instruction = 4x fp32r
  throughput) for the QKV projection, attn@V (ctx), and Wo.  Scores and the
  FFN run in bf16 (1 cycle/row).  Wq/Wo are pre-scaled by 32 on the host for
  fp8 representability; the 1/32 is folded into the PSUM->SBUF copies.
- Softmax without max-subtraction: q=k=v here (source repo bug reproduced in
  the reference), scores are bounded; exp(s/8)/8 fits fp8e4m3.  Masking is
  applied to V and to the appended ones-column that yields Z, so
  ctx = (V^T T)/Z equals the reference softmax exactly.
- No PE transposes: x arrives pre-transposed+quantized from the host; p^T/h1^T
  are transposed via the DMA XBAR (bf16), which runs on the DMA engines.
- Host folds: g1 into W1, b1@W1 into bf1, bo into the residual copy of x,
  b1+bf2 into the h1 residual bias.
- LayerNorm scale-invariance: the Wo output PSUM (32x-scaled) is added to a
  host-precomputed 32*(x+bo) residual and normalized directly — the 1/32
  never needs to be applied.
- Engine balance/overlap: the exp stream on ACT paces attention while PE/DVE/
  Pool run scores/ctx/normalize under it; pT quantize on ACT, pkT on DVE;
  Phase-C LayerNorm (DVE) is overlapped with FFN1 chunks on PE; critical DMAs
  issue from SP, small broadcasts from Pool (SWDGE); LN applies run on DVE in
  bf16 (2x/4x DVE perf modes).
"""

import numpy as np
import ml_dtypes

import concourse.bass as bass
import concourse.tile as tile
from concourse import bacc
from concourse import mybir
from concourse.bass_utils import run_bass_kernel_spmd

B, S, D, H, F = 8, 1024, 768, 12, 3072
DK = D // H          # 64
P = 128
QT = S // P          # 8 query tiles
DT = D // P          # 6 d tiles
FT = F // P          # 24 f tiles
EPS = 1e-5
WSCALE = 32.0        # host pre-scale on Wq/Wo for fp8 representability
LN8 = 2.0794415416798357  # ln(8); exp(s/8)/8 = exp(s*0.125 - LN8)

f32 = mybir.dt.float32
bf16 = mybir.dt.bfloat16
f8 = mybir.dt.float8e4
AF = mybir.ActivationFunctionType
ALU = mybir.AluOpType

np_bf16 = ml_dtypes.bfloat16
np_f8 = ml_dtypes.float8_e4m3


def _pbc(src_ap, nparts):
    """Partition-broadcast access pattern: [N] dram -> [nparts, N] with step 0."""
    return bass.AP(
        tensor=src_ap.tensor, offset=src_ap.offset, ap=[[0, nparts], src_ap.ap[-1]]
    )


def build_bass(NK=640):
    nc = bacc.Bacc()

    xt8_d = nc.dram_tensor("xT8", [D, S], f8, kind="ExternalInput")
    xkt8_d = nc.dram_tensor("xkT8", [D, NK], f8, kind="ExternalInput")
    x2_d = nc.dram_tensor("x2", [S, D], bf16, kind="ExternalInput")
    kmask_d = nc.dram_tensor("kmask", [NK], f32, kind="ExternalInput")
    wq_d = nc.dram_tensor("Wq8", [D, D], f8, kind="ExternalInput")
    bq_d = nc.dram_tensor("bq", [D], f32, kind="ExternalInput")
    wo_d = nc.dram_tensor("Wo8", [D, D], f8, kind="ExternalInput")
    w1_d = nc.dram_tensor("W1b", [D, F], bf16, kind="ExternalInput")
    bf1_d = nc.dram_tensor("bf1p", [F], f32, kind="ExternalInput")
    w2_d = nc.dram_tensor("W2b", [F, D], bf16, kind="ExternalInput")
    g1_d = nc.dram_tensor("g1b", [D], bf16, kind="ExternalInput")
    b1_d = nc.dram_tensor("b1f2b", [D], bf16, kind="ExternalInput")
    g2_d = nc.dram_tensor("g2", [D], f32, kind="ExternalInput")
    b2_d = nc.dram_tensor("b2", [D], f32, kind="ExternalInput")
    y_d = nc.dram_tensor("y", [S, D], f32, kind="ExternalOutput")

    with tile.TileContext(nc) as tc:
        _emit(tc, NK, xt8_d, xkt8_d, x2_d, kmask_d, wq_d, bq_d, wo_d,
              w1_d, bf1_d, w2_d, g1_d, b1_d, g2_d, b2_d, y_d)
    nc.compile()
    return nc


def _emit(tc, NK, xt8_d, xkt8_d, x2_d, kmask_d, wq_d, bq_d, wo_d,
          w1_d, bf1_d, w2_d, g1_d, b1_d, g2_d, b2_d, y_d):
    nc = tc.nc
    from contextlib import ExitStack

    KT = NK // P         # key tiles (real)
    NPAIR = DT // 2      # d-tile pairs for DoubleRow
    DR = mybir.MatmulPerfMode.DoubleRow

    with ExitStack() as ctx:
        singles = ctx.enter_context(tc.tile_pool(name="singles", bufs=1, side="left"))

        eps_t = singles.tile([P, 1], f32, tag="eps")
        nc.vector.memset(eps_t, EPS)
        ln8_t = singles.tile([P, 1], f32, tag="ln8")
        nc.vector.memset(ln8_t, -LN8)

        # critical-path loads on SP; the rest issued from Pool (SWDGE) so the
        # SP queue reaches the projection weights immediately
        bqcol = singles.tile([P, DT], f32, tag="bqcol")
        nc.sync.dma_start(bqcol, bq_d[:].rearrange("(t p) -> p t", p=P))
        kmaskcol = singles.tile([P, KT], f32, tag="kmc")
        nc.sync.dma_start(kmaskcol, kmask_d[:].rearrange("(t p) -> p t", p=P))
        bf1col = singles.tile([P, FT], f32, tag="bf1col")

        g1b = singles.tile([P, D], bf16, tag="g1b")
        nc.gpsimd.dma_start(g1b, _pbc(g1_d[:], P))
        b1f2b = singles.tile([P, D], bf16, tag="b1f2b")
        nc.gpsimd.dma_start(b1f2b, _pbc(b1_d[:], P))
        g2b = singles.tile([P, D], f32, tag="g2b")
        b2b = singles.tile([P, D], f32, tag="b2b")

        # ---- persistent pools, allocated bottom-of-stack first (LIFO) ----
        # left stack: singles | h1 | x2 | wo | pT | vaug | T8 | wq | xt | pkrows
        h1_pool = tc.alloc_tile_pool(name="h1p", bufs=1, side="left")
        h1p = [h1_pool.tile([P, D], bf16, tag=f"h1p{q}", name=f"h1p{q}")
               for q in range(QT)]
        h1cT = h1_pool.tile([P, DT, S], bf16, tag="h1cT", name="h1cT")

        x2_pool = tc.alloc_tile_pool(name="x2p", bufs=1, side="left")
        x2 = [x2_pool.tile([P, D], bf16, tag=f"x2{q}", name=f"x2{q}")
              for q in range(QT)]


        wo_pool = tc.alloc_tile_pool(name="wop", bufs=1, side="left")
        wo8 = [wo_pool.tile([P, 2, D], f8, tag=f"wo{p}", name=f"wo{p}")
               for p in range(NPAIR)]

        pt_pool = tc.alloc_tile_pool(name="ptp", bufs=1, side="left")
        pT = [pt_pool.tile([P, S], bf16, tag=f"pT{d}", name=f"pT{d}")
              for d in range(DT)]
        pkT = [pt_pool.tile([P, NK], bf16, tag=f"pkT{d}", name=f"pkT{d}")
               for d in range(DT)]

        # per-head stride padded 65->68 so the DoubleRow k-pair stride
        # (H*68=816 bytes) meets the 16B ISA alignment requirement
        VS = 68
        vg_pool = tc.alloc_tile_pool(name="vgp", bufs=1, side="left")
        vaug = vg_pool.tile([P, KT, H, VS], f8, tag="vaug", name="vaug")

        t8_pool = tc.alloc_tile_pool(name="t8p", bufs=1, side="left")
        T8 = [t8_pool.tile([P, KT, S], f8, tag=f"T8{i}", name=f"T8{i}")
              for i in range(3)]

        wq_pool = tc.alloc_tile_pool(name="wqp", bufs=1, side="left")
        wq8 = [wq_pool.tile([P, 2, D], f8, tag=f"wq{p}", name=f"wq{p}")
               for p in range(NPAIR)]
        for p in range(NPAIR):
            nc.sync.dma_start(
                wq8[p],
                wq_d[2 * p * P: (2 * p + 2) * P, :].rearrange(
                    "(two p) d -> p two d", p=P),
            )

        xt_pool = tc.alloc_tile_pool(name="xtp", bufs=1, side="left")
        xt8 = [xt_pool.tile([P, 2, S], f8, tag=f"xt{p}", name=f"xt{p}")
               for p in range(NPAIR)]
        for p in range(NPAIR):
            nc.sync.dma_start(
                xt8[p],
                xt8_d[2 * p * P: (2 * p + 2) * P, :].rearrange(
                    "(two p) q -> p two q", p=P),
            )
        xkt8 = [xt_pool.tile([P, 2, NK], f8, tag=f"xk{p}", name=f"xk{p}")
                for p in range(NPAIR)]
        for p in range(NPAIR):
            nc.sync.dma_start(
                xkt8[p],
                xkt8_d[2 * p * P: (2 * p + 2) * P, :].rearrange(
                    "(two p) k -> p two k", p=P),
            )

        pkr_pool = tc.alloc_tile_pool(name="pkrp", bufs=1, side="left")
        pkrows = [pkr_pool.tile([P, KT, P], bf16, tag=f"pkr{d}", name=f"pkr{d}")
                  for d in range(DT)]

        # right stack: w1 | w2 | ctxT8 | gT(qh0) | gT(qh1)
        w1_pool = tc.alloc_tile_pool(name="w1p", bufs=1, side="right")
        w1 = [w1_pool.tile([P, F], bf16, tag=f"w1{d}", name=f"w1{d}")
              for d in range(DT)]

        w2_pool = tc.alloc_tile_pool(name="w2p", bufs=1, side="right")
        w2 = [w2_pool.tile([P, D], bf16, tag=f"w2{t}", name=f"w2{t}")
              for t in range(FT)]

        ctx_pool = tc.alloc_tile_pool(name="ctxp", bufs=1, side="right")
        ctxT8 = [ctx_pool.tile([P, 2, S], f8, tag=f"cx{p}", name=f"cx{p}")
                 for p in range(NPAIR)]

        # ---- Phase A: projections (fp8 DoubleRow), pT/pkT in bf16 ----
        # Per do-tile: proj-k then proj-q, PSUM->SBUF quantize on DVE, XBAR
        # transpose + vaug masking as soon as that do-tile's pkT lands, so
        # attention head 0 can start while later do-tiles still project.
        with tc.tile_pool(name="psA", bufs=2, space="PSUM") as psA:
            for do in range(DT):
                ps = psA.tile([P, NK], f32, tag="pk")
                for oc, osz in ((0, 512), (512, NK - 512)):
                    for p in range(NPAIR):
                        nc.tensor.matmul(
                            ps[:, oc: oc + osz],
                            wq8[p][:, :, do * P: (do + 1) * P],
                            xkt8[p][:, :, oc: oc + osz],
                            start=(p == 0), stop=(p == NPAIR - 1),
                            perf_mode=DR,
                        )
                nc.vector.tensor_scalar(
                    pkT[do], ps, 1.0 / WSCALE, bqcol[:, do: do + 1],
                    op0=ALU.mult, op1=ALU.add,
                )
                nc.sync.dma_start_transpose(pkrows[do], pkT[do])
                ps = psA.tile([P, S], f32, tag="pj")
                for qc in range(2):
                    for p in range(NPAIR):
                        nc.tensor.matmul(
                            ps[:, qc * 512: (qc + 1) * 512],
                            wq8[p][:, :, do * P: (do + 1) * P],
                            xt8[p][:, :, qc * 512: (qc + 1) * 512],
                            start=(p == 0), stop=(p == NPAIR - 1),
                            perf_mode=DR,
                        )
                nc.scalar.activation(
                    pT[do], ps, AF.Identity,
                    bias=bqcol[:, do: do + 1], scale=1.0 / WSCALE,
                )
                for kt in range(KT):
                    nc.vector.tensor_scalar_mul(
                        vaug[:, kt, 2 * do: 2 * do + 2, 0:DK],
                        pkrows[do][:, kt, :].rearrange("p (h e) -> p h e", h=2),
                        kmaskcol[:, kt: kt + 1],
                    )
        for kt in range(KT):
            nc.vector.tensor_copy(
                vaug[:, kt, :, DK: DK + 1],
                kmaskcol[:, kt: kt + 1, None].to_broadcast((P, H, 1)),
            )
        for d in range(DT):
            nc.sync.dma_start(w1[d], w1_d[d * P: (d + 1) * P, :])
        for q in range(QT):
            nc.sync.dma_start(x2[q], x2_d[q * P: (q + 1) * P, :])
        for t in range(FT):
            nc.sync.dma_start(w2[t], w2_d[t * P: (t + 1) * P, :])
        pkr_pool.release()
        xt_pool.release()
        wq_pool.release()

        # gelu outputs, one pool per q-half so the second can reuse the
        # released projection-weight space
        gt0_pool = tc.alloc_tile_pool(name="gt0p", bufs=1, side="right")
        gT0 = [gt0_pool.tile([P, 512], bf16, tag=f"g0_{t}", name=f"g0_{t}")
               for t in range(FT)]

        # ---- Phase B: attention (exp on ACT is the pacer; PE, DVE and Pool
        # run the scores/ctx/normalize chain underneath it) ----
        def ln_stats(pool, z, tag):
            st = pool.tile([P, 2, 6], f32, tag=f"bnst{tag}")
            for sg in range(2):
                nc.vector.bn_stats(st[:, sg, :], z[:, sg * 384: (sg + 1) * 384])
            mv = pool.tile([P, 2], f32, tag=f"bnmv{tag}")
            nc.vector.bn_aggr(mv, st)
            return mv

        with tc.tile_pool(name="psS", bufs=3, space="PSUM") as psS, \
             tc.tile_pool(name="psC", bufs=2, space="PSUM") as psC, \
             tc.tile_pool(name="zpool", bufs=2) as zp:
            for h in range(H):
                if h == 0:
                    # non-critical loads issued from Pool behind the memsets
                    for p in range(NPAIR):
                        nc.gpsimd.dma_start(
                            wo8[p],
                            wo_d[2 * p * P: (2 * p + 2) * P, :].rearrange(
                                "(two p) d -> p two d", p=P),
                        )
                    nc.gpsimd.dma_start(
                        bf1col, bf1_d[:].rearrange("(t p) -> p t", p=P))
                    nc.gpsimd.dma_start(g2b, _pbc(g2_d[:], P))
                    nc.gpsimd.dma_start(b2b, _pbc(b2_d[:], P))
                t8 = T8[h % 3]
                do, hh = h // 2, h % 2
                for kt in range(KT):
                    ps = psS.tile([P, S], f32, tag="sc")
                    for qh in range(2):
                        nc.tensor.matmul(
                            ps[:, qh * 512: (qh + 1) * 512],
                            pkT[do][hh * DK: (hh + 1) * DK,
                                    kt * P: (kt + 1) * P],
                            pT[do][hh * DK: (hh + 1) * DK,
                                   qh * 512: (qh + 1) * 512],
                            start=True, stop=True,
                            tile_position=(hh * DK, 0),
                        )
                    nc.scalar.activation(
                        t8[:, kt, :], ps, AF.Exp, bias=ln8_t, scale=0.125,
                    )
                p4, s2 = h // 4, (h // 2) % 2
                for qh in range(2):
                    cps = psC.tile([DK + 1, 512], f32, tag="cx")
                    for j in range((KT - 1) // 2):
                        nc.tensor.matmul(
                            cps,
                            vaug[:, 2 * j: 2 * j + 2, h, 0: DK + 1],
                            t8[:, 2 * j: 2 * j + 2,
                               qh * 512: (qh + 1) * 512],
                            start=(j == 0), stop=False,
                            perf_mode=DR,
                        )
                    if KT % 2:
                        nc.tensor.matmul(
                            cps,
                            vaug[:, KT - 1, h, 0: DK + 1],
                            t8[:, KT - 1, qh * 512: (qh + 1) * 512],
                            start=False, stop=True,
                        )
                    else:
                        nc.tensor.matmul(
                            cps,
                            vaug[:, KT - 2: KT, h, 0: DK + 1],
                            t8[:, KT - 2: KT, qh * 512: (qh + 1) * 512],
                            start=False, stop=True, perf_mode=DR,
                        )
                    zrow = zp.tile([1, 512], f32, tag="zrow")
                    nc.vector.reciprocal(zrow, cps[DK: DK + 1, :])
                    invb = zp.tile([DK, 512], f32, tag="invb")
                    nc.gpsimd.partition_broadcast(invb, zrow)
                    nc.vector.tensor_mul(
                        ctxT8[p4][hh * DK: (hh + 1) * DK, s2,
                                  qh * 512: (qh + 1) * 512],
                        cps[0:DK, :],
                        invb,
                    )
        t8_pool.release()
        vg_pool.release()
        pt_pool.release()

        # ---- Phase C: Wo (fp8 DR), residual, LN1 — pipelined per qt-pair,
        # with FFN1 chunks of the finished q-half interleaved so PE works
        # while the DVE LayerNorm chain runs ----
        def ffn1(ft, qh, psF1, gt):
            ps = psF1.tile([P, 512], f32, tag="f1")
            for d in range(DT):
                nc.tensor.matmul(
                    ps,
                    w1[d][:, ft * P: (ft + 1) * P],
                    h1cT[:, d, qh * 512: (qh + 1) * 512],
                    start=(d == 0), stop=(d == DT - 1),
                )
            nc.scalar.activation(
                gt[ft], ps, AF.Gelu, bias=bf1col[:, ft: ft + 1],
            )

        with tc.tile_pool(name="pC", bufs=2) as pC, \
             tc.tile_pool(name="pCz", bufs=2) as pCz, \
             tc.tile_pool(name="psW", bufs=2, space="PSUM") as psW, \
             tc.tile_pool(name="psF1", bufs=4, space="PSUM") as psF1:
            for pair in range(QT // 2):
                for qi in range(2):
                    qt = 2 * pair + qi
                    ps = psW.tile([P, D], f32, tag="wo")
                    for oc, osz in ((0, 512), (512, 256)):
                        for p in range(NPAIR):
                            nc.tensor.matmul(
                                ps[:, oc: oc + osz],
                                ctxT8[p][:, :, qt * P: (qt + 1) * P],
                                wo8[p][:, :, oc: oc + osz],
                                start=(p == 0), stop=(p == NPAIR - 1),
                                perf_mode=DR,
                            )
                    # LN is scale-invariant: z1' = Wo8-psum + 32*(x+bo)
                    # normalizes identically to z1 = psum/32 + x + bo.
                    # Row sums come free from the add (accum_out); the second
                    # moment runs on ACT (Square+accum, in every ACT table),
                    # keeping the DVE chain short in this DVE-bound window.
                    z1 = pCz.tile([P, D], bf16, tag=f"z1_{qi}")
                    zsum = pCz.tile([P, 1], f32, tag=f"zs{qi}")
                    nc.vector.scalar_tensor_tensor(
                        z1, ps, 1.0, x2[qt], op0=ALU.mult, op1=ALU.add,
                        accum_out=zsum,
                    )
                    scr = pC.tile([P, D], bf16, tag=f"scr{qi}")
                    sqacc = pCz.tile([P, 1], f32, tag=f"sq{qi}")
                    nc.scalar.activation(scr, z1, AF.Square, accum_out=sqacc)
                    mu = pCz.tile([P, 1], f32, tag=f"mu{qi}")
                    nc.vector.tensor_scalar_mul(mu, zsum, 1.0 / D)
                    nmu = pCz.tile([P, 1], f32, tag=f"nmu{qi}")
                    nc.vector.tensor_scalar_mul(nmu, zsum, -1.0 / D)
                    nme = pCz.tile([P, 1], f32, tag=f"nme{qi}")
                    nc.vector.tensor_scalar(
                        nme, mu, nmu, EPS, op0=ALU.mult, op1=ALU.add,
                    )
                    std1 = pCz.tile([P, 1], f32, tag=f"std{qi}")
                    nc.scalar.activation(std1, sqacc, AF.Sqrt, bias=nme,
                                         scale=1.0 / D)
                    rstd1 = pCz.tile([P, 1], f32, tag=f"rstd{qi}")
                    nc.vector.reciprocal(rstd1, std1)
                    h1c = pC.tile([P, D], bf16, tag=f"h1c{qi}")
                    nc.vector.tensor_scalar(
                        h1c, z1, mu, rstd1,
                        op0=ALU.subtract, op1=ALU.mult,
                    )
                    nc.sync.dma_start_transpose(
                        h1cT[:, :, qt * P: (qt + 1) * P], h1c)
                    nc.vector.tensor_mul(h1p[qt], h1c, g1b)
                    nc.vector.tensor_add(h1p[qt], h1p[qt], b1f2b)
                if pair == 1:
                    for ft in range(8):
                        ffn1(ft, 0, psF1, gT0)
                elif pair == 2:
                    for ft in range(8, 16):
                        ffn1(ft, 0, psF1, gT0)
                elif pair == 3:
                    for ft in range(16, FT):
                        ffn1(ft, 0, psF1, gT0)

        # ---- Phase D: rest of the FFN (bf16) + LN2 by qt-pair ----
        gt1_pool = tc.alloc_tile_pool(name="gt1p", bufs=1, side="right")
        gT1 = [gt1_pool.tile([P, 512], bf16, tag=f"g1_{t}", name=f"g1_{t}")
               for t in range(FT)]
        with tc.tile_pool(name="psF1b", bufs=4, space="PSUM") as psF1b, \
             tc.tile_pool(name="pD", bufs=2) as pD, \
             tc.tile_pool(name="pDz", bufs=2) as pDz, \
             tc.tile_pool(name="psF2", bufs=2, space="PSUM") as psF2:
            for ft in range(FT):
                ffn1(ft, 1, psF1b, gT1)

            for qt in range(QT):
                gt, qo = (gT0, 0) if qt < 4 else (gT1, 4)
                # separate PSUM tiles per column group so the z2 chain can
                # start on the first group while the second still accumulates
                psa = psF2.tile([P, 512], f32, tag="f2a")
                psb = psF2.tile([P, 256], f32, tag="f2b")
                for ps_t, oc, osz in ((psa, 0, 512), (psb, 512, 256)):
                    for ft in range(FT):
                        nc.tensor.matmul(
                            ps_t,
                            gt[ft][:, (qt - qo) * P: (qt - qo + 1) * P],
                            w2[ft][:, oc: oc + osz],
                            start=(ft == 0), stop=(ft == FT - 1),
                        )
                z2 = pDz.tile([P, D], bf16, tag="z2")
                nc.vector.tensor_add(z2[:, 0:512], psa, h1p[qt][:, 0:512])
                st = pDz.tile([P, 2, 6], f32, tag="bnst0")
                nc.vector.bn_stats(st[:, 0, :], z2[:, 0:384])
                nc.vector.tensor_add(z2[:, 512:D], psb, h1p[qt][:, 512:D])
                nc.vector.bn_stats(st[:, 1, :], z2[:, 384:768])
                mv = pDz.tile([P, 2], f32, tag="bnmv0")
                nc.vector.bn_aggr(mv, st)
                std1 = pDz.tile([P, 1], f32, tag="std1")
                nc.scalar.activation(std1, mv[:, 1:2], AF.Sqrt, bias=eps_t)
                rstd1 = pDz.tile([P, 1], f32, tag="rstd1")
                nc.vector.reciprocal(rstd1, std1)
                t0 = pD.tile([P, D], f32, tag="t0")
                yt = pD.tile([P, D], f32, tag="yt")
                for oc, osz in ((0, 384), (384, 384)):
                    sl = slice(oc, oc + osz)
                    nc.vector.tensor_scalar(
                        t0[:, sl], z2[:, sl], mv[:, 0:1], rstd1,
                        op0=ALU.subtract, op1=ALU.mult,
                    )
                    nc.vector.tensor_mul(yt[:, sl], t0[:, sl], g2b[:, sl])
                    nc.vector.tensor_add(yt[:, sl], yt[:, sl], b2b[:, sl])
                    nc.sync.dma_start(
                        y_d[qt * P: (qt + 1) * P, sl], yt[:, sl])
        gt1_pool.release()
        gt0_pool.release()
        ctx_pool.release()
        w2_pool.release()
        w1_pool.release()
        wo_pool.release()
        x2_pool.release()
        h1_pool.release()


_BASS_CACHE = {}


def _get_bass(NK=640):
    if NK not in _BASS_CACHE:
        _BASS_CACHE[NK] = build_bass(NK)
    return _BASS_CACHE[NK]


def prep_inputs(inputs):
    """Host-side preprocessing: gather keys, quantize, fold constants."""
    x = np.asarray(inputs["x"], np.float32)
    mask = np.asarray(inputs["attn_mask"]).astype(np.int64)
    Wq = np.asarray(inputs["Wq"], np.float32)
    bq = np.asarray(inputs["bq"], np.float32)
    Wo = np.asarray(inputs["Wo"], np.float32)
    bo = np.asarray(inputs["bo"], np.float32)
    g1 = np.asarray(inputs["g1"], np.float32)
    b1 = np.asarray(inputs["b1"], np.float32)
    W1 = np.asarray(inputs["W1"], np.float32)
    bf1 = np.asarray(inputs["bf1"], np.float32)
    W2 = np.asarray(inputs["W2"], np.float32)
    bf2 = np.asarray(inputs["bf2"], np.float32)
    g2 = np.asarray(inputs["g2"], np.float32)
    b2 = np.asarray(inputs["b2"], np.float32)

    nk_max = int(mask.sum(axis=1).max())
    NK = max(640, -(-nk_max // P) * P)

    shared = {
        "Wq8": np.ascontiguousarray((Wq * WSCALE).astype(np_f8)),
        "bq": bq,
        "Wo8": np.ascontiguousarray((Wo * WSCALE).astype(np_f8)),
        "W1b": np.ascontiguousarray((g1[:, None] * W1).astype(np_bf16)),
        "bf1p": np.ascontiguousarray(bf1 + b1 @ W1),
        "W2b": np.ascontiguousarray(W2.astype(np_bf16)),
        "g1b": g1.astype(np_bf16),
        "b1f2b": (b1 + bf2).astype(np_bf16),
        "g2": g2,
        "b2": b2,
    }
    in_maps = []
    for b in range(B):
        xb = x[b]
        idx = np.nonzero(mask[b])[0]
        nk = len(idx)
        xk = np.zeros((NK, D), np.float32)
        xk[:nk] = xb[idx]
        kmask = np.zeros((NK,), np.float32)
        kmask[:nk] = 1.0
        m = {
            "xT8": np.ascontiguousarray(xb.T.astype(np_f8)),
            "xkT8": np.ascontiguousarray(xk.T.astype(np_f8)),
            "x2": np.ascontiguousarray((WSCALE * (xb + bo)).astype(np_bf16)),
            "kmask": kmask,
        }
        m.update(shared)
        in_maps.append(m)
    return NK, in_maps


def kernel(**inputs):
    NK, in_maps = prep_inputs(inputs)
    nc = _get_bass(NK)
    res = run_bass_kernel_spmd(nc, in_maps, core_ids=list(range(B)))
    return np.stack([res.results[b]["y"] for b in range(B)], axis=0).astype(
        np.float32)


if __name__ == "__main__":
    nc = build_bass()
    print("bass build ok")
